# revision 17
# baseline (speedup 1.0000x reference)
"""GCN encoder (2-layer) on 8 trn2 NeuronCores via Bass/Tile.

Strategy (sharding_hint: 1D partition nodes by destination):
  - Nodes sharded by destination: core c owns rows [c*12500, (c+1)*12500).
  - Per layer, messages are gathered per-edge with dma_gather (SWDGE) from a
    full replica of the (scaled) source features in HBM, combined into
    per-destination sums with PE matmuls against one-hot selection matrices
    (built on DVE with is_equal), then the dense 128x128 GEMM runs on the
    aggregated features (GEMM commutes with the segment-sum).
  - Layer 1 sources = dinv-scaled x (host-prepared, replicated, no comms).
    Layer 2 sources = dinv-scaled h1, exchanged with ONE AllGather.
  - dma_gather indices are int16, so sources are split into 4 banks
    (32768, 32768, 32768, 1696 rows); edges are grouped per (dest-tile, bank)
    and padded to 128-edge chunks, with chunk counts maxed over cores so the
    same program (SPMD) runs on all 8 cores.
"""

import sys
import numpy as np

sys.path.insert(0, "/opt/trn_rl_repo")
sys.path.insert(0, "/opt/trn_rl_repo/concourse")

N, E, D = 100000, 1600000, 128
NC = 8
NLOC = N // NC            # 12500 rows per core
P = 128
NT = (NLOC + P - 1) // P  # 98 dest tiles per core (last tile 84 rows)
REG = 4                   # dest tiles per gather region
BANK_BOUNDS = [0, 32768, 65536, 98304, N]
NBANK = 4

_cache = {}


def _schedule(edge_index):
    """Compute the cross-core-uniform chunk schedule and per-core arrays."""
    row = edge_index[0].astype(np.int64)
    col = edge_index[1].astype(np.int64)
    loop = np.arange(N, dtype=np.int64)
    row = np.concatenate([row, loop])
    col = np.concatenate([col, loop])

    deg = np.bincount(col, minlength=N).astype(np.float32)
    dinv = np.where(deg > 0, 1.0 / np.sqrt(deg), 0.0).astype(np.float32)

    core = col // NLOC
    tile = (col % NLOC) // P
    bank = np.searchsorted(BANK_BOUNDS, row, side="right") - 1
    key = (core * NT + tile) * NBANK + bank
    order = np.argsort(key, kind="stable")
    row_s, col_s, key_s = row[order], col[order], key[order]

    counts = np.bincount(key_s, minlength=NC * NT * NBANK).reshape(NC, NT, NBANK)
    K = np.ceil(counts.max(axis=0) / P).astype(np.int64)      # [NT, NBANK]
    K[:, 0] = np.maximum(K[:, 0], 1)                           # every tile has >=1 chunk

    # chunk columns in matmul order: (region, tile, bank, j)
    col_base = np.zeros((NT, NBANK), np.int64)
    c_total = 0
    for t in range(NT):
        for b in range(NBANK):
            col_base[t, b] = c_total
            c_total += K[t, b]

    # gather-call layout: per region r, per bank b, chunks for tiles of r in
    # t order.  slot_base[t, b] = slot index (in 128-edge units) of chunk 0 of
    # (t, b) within the global call stream.
    regions = [list(range(r, min(r + REG, NT))) for r in range(0, NT, REG)]
    CALL_CH = 12              # chunks per dma_gather sub-call (1536 indices)
    region_calls = []         # per region: list of (bank, slot_chunk_lo, n_chunks)
    slot_base = np.zeros((NT, NBANK), np.int64)
    stream_pos = 0
    for tiles in regions:
        rc = []
        for b in range(NBANK):
            n_ch = int(sum(K[t, b] for t in tiles))
            if n_ch == 0:
                continue
            for t in tiles:
                slot_base[t, b] = stream_pos
                stream_pos += K[t, b]
            lo = stream_pos - n_ch
            # split into uniform sub-calls so SWDGE queue contexts pipeline
            nsub = max(1, (n_ch + CALL_CH - 1) // CALL_CH)
            base = n_ch // nsub
            rem = n_ch - base * nsub
            off = lo
            for si in range(nsub):
                cnt = base + (1 if si < rem else 0)
                rc.append((b, off, cnt))
                off += cnt
            assert off == stream_pos
        region_calls.append(rc)
    assert stream_pos == c_total

    tot_slots = c_total * P

    # per-core arrays
    group_cnt = counts  # [NC, NT, NBANK]
    group_off = np.zeros(NC * NT * NBANK + 1, np.int64)
    np.cumsum(np.bincount(key_s, minlength=NC * NT * NBANK), out=group_off[1:])

    import ml_dtypes
    idx_list, s_list = [], []
    for c in range(NC):
        idx16 = np.zeros((P, tot_slots // 16), np.int16)
        s_flat = np.zeros(P * c_total * P, np.uint8)  # fp8 one-hot, built flat
        one_fp8 = np.float32(1.0).astype(ml_dtypes.float8_e4m3).view(np.uint8)
        for t in range(NT):
            for b in range(NBANK):
                g = (c * NT + t) * NBANK + b
                lo, hi = group_off[g], group_off[g + 1]
                n = hi - lo
                if n == 0 and K[t, b] == 0:
                    continue
                pos = np.arange(n)
                j = pos // P
                p = pos % P
                # matmul-order one-hot: S[p, (col_base+j)*128 + dloc] = 1
                cols = col_base[t, b] + j
                dloc = col_s[lo:hi] - (c * NLOC + t * P)
                s_flat[(p * c_total + cols) * P + dloc] = one_fp8
                # gather stream (idx pads are 0 -> harmless row-0 gathers)
                slot = (slot_base[t, b] + j) * P + p
                v = (row_s[lo:hi] - BANK_BOUNDS[b]).astype(np.int16)
                idx16[slot % 16, slot // 16] = v
        idx16 = np.tile(idx16[:16], (8, 1))
        idx_list.append(idx16)
        s_list.append(s_flat.view(ml_dtypes.float8_e4m3).reshape(P, c_total * P))

    dinvp = np.zeros((P, NT), np.float32)
    dinvp_all = []
    for c in range(NC):
        dp = np.zeros((P, NT), np.float32)
        dl = dinv[c * NLOC:(c + 1) * NLOC]
        dl = np.pad(dl, (0, NT * P - NLOC))
        dp[:, :] = dl.reshape(NT, P).T
        dinvp_all.append(dp)
    del dinvp

    sched = {
        "K": K, "col_base": col_base, "slot_base": slot_base,
        "region_calls": region_calls, "regions": regions,
        "c_total": c_total, "tot_slots": tot_slots,
    }
    percore = {
        "idx": idx_list, "s": s_list, "dinvp": dinvp_all,
    }
    return sched, percore, dinv


def _build_program(sched):
    import os
    from concourse import bass, bacc, mybir
    import concourse.tile as tile

    ablate = os.environ.get("GCN_ABLATE", "")

    fp16 = mybir.dt.float16
    f32 = mybir.dt.float32
    i16 = mybir.dt.int16

    K = sched["K"]
    col_base = sched["col_base"]
    slot_base = sched["slot_base"]
    region_calls = sched["region_calls"]
    regions = sched["regions"]
    c_total = sched["c_total"]
    tot_slots = sched["tot_slots"]

    nc = bacc.Bacc("TRN2", target_bir_lowering=False, debug=False,
                   num_devices=NC, num_swdge_queues=4)

    xs = nc.declare_dram_parameter("xs", [N, D], fp16, isOutput=False)
    idx = nc.declare_dram_parameter("idx", [P, tot_slots // 16], i16, isOutput=False)
    s_d = nc.declare_dram_parameter("s8", [P, c_total * P], mybir.dt.float8e4,
                                    isOutput=False)
    dinvp_d = nc.declare_dram_parameter("dinvp", [P, NT], f32, isOutput=False)
    w1_d = nc.declare_dram_parameter("w1", [D, D], fp16, isOutput=False)
    w2_d = nc.declare_dram_parameter("w2", [D, D], fp16, isOutput=False)
    b1_d = nc.declare_dram_parameter("b1", [P, D], f32, isOutput=False)
    b2_d = nc.declare_dram_parameter("b2", [P, D], f32, isOutput=False)
    out_d = nc.declare_dram_parameter("out", [NLOC, 2 * D], f32, isOutput=True)

    h1s_loc = nc.dram_tensor("h1s_loc", [NLOC, D], fp16)
    h1s_full = nc.dram_tensor("h1s_full", [N, D], fp16, addr_space="Shared")

    with tile.TileContext(nc) as tc:
        with tc.tile_pool(name="const", bufs=1) as cpool, \
             tc.tile_pool(name="msg", bufs=4) as mpool, \
             tc.tile_pool(name="sel", bufs=3) as spool, \
             tc.tile_pool(name="epi", bufs=3) as epool, \
             tc.tile_pool(name="acc", bufs=3, space="PSUM") as accp, \
             tc.tile_pool(name="gemm", bufs=2, space="PSUM") as gemp:

            idx_sb = cpool.tile([P, tot_slots // 16], i16)
            nc.sync.dma_start(out=idx_sb[:], in_=idx[:, :])
            dinvp_sb = cpool.tile([P, NT], f32)
            nc.sync.dma_start(out=dinvp_sb[:], in_=dinvp_d[:, :])
            w_sb = [cpool.tile([P, D], fp16, name="w1sb"),
                    cpool.tile([P, D], fp16, name="w2sb")]
            nc.sync.dma_start(out=w_sb[0][:], in_=w1_d[:, :])
            nc.sync.dma_start(out=w_sb[1][:], in_=w2_d[:, :])
            b_sb = [cpool.tile([P, D], f32, name="b1sb"),
                    cpool.tile([P, D], f32, name="b2sb")]
            nc.sync.dma_start(out=b_sb[0][:], in_=b1_d[:, :])
            nc.sync.dma_start(out=b_sb[1][:], in_=b2_d[:, :])

            qi = [0]

            def layer(li, src):
                for ri, tiles in enumerate(regions):
                    # load this region's selection matrices (fp8 one-hot)
                    rch0 = int(col_base[tiles[0], 0])
                    rchn = int(sum(K[t].sum() for t in tiles))
                    s_sb = spool.tile([P, rchn * P], mybir.dt.float8e4, tag="S")
                    nc.sync.dma_start(out=s_sb[:],
                                      in_=s_d[:, rch0 * P:(rch0 + rchn) * P])
                    # gather this region's chunks (uniform sub-calls,
                    # strict queue round-robin so the 4 SWDGE contexts
                    # generate descriptors in parallel)
                    bufs = {}
                    bank_lo = {}
                    for b, s_lo, n_ch in region_calls[ri]:
                        if b not in bufs:
                            tot_b = sum(c for bb, _, c in region_calls[ri]
                                        if bb == b)
                            bufs[b] = mpool.tile([P, tot_b * D], fp16,
                                                 name=f"msgb{b}",
                                                 tag=f"msg{b}")
                            bank_lo[b] = s_lo
                        m = bufs[b]
                        mo = s_lo - bank_lo[b]
                        nc.gpsimd.dma_gather(
                            out_ap=m[:, mo * D:(mo + n_ch) * D].rearrange(
                                "p (c f) -> p c f", f=D),
                            in_ap=src[BANK_BOUNDS[b]:BANK_BOUNDS[b + 1], :],
                            idxs_ap=idx_sb[:, s_lo * 8:(s_lo + n_ch) * 8],
                            num_idxs=n_ch * P,
                            num_idxs_reg=n_ch * P,
                            elem_size=D,
                            single_packet=False,
                            queue_num=1 + qi[0] % 3,
                        )
                        qi[0] += 1
                    bufs = {b: (m, bank_lo[b]) for b, m in bufs.items()}

                    if ablate == "gather":
                        # consume msg buffers minimally, skip seg/epilogue
                        for b, (m, s_lo) in bufs.items():
                            o32 = epool.tile([P, P], f32, tag="o32")
                            nc.vector.tensor_copy(out=o32[:], in_=m[:, :P])
                            rows = min(P, NLOC - tiles[0] * P)
                            nc.sync.dma_start(
                                out=out_d[tiles[0] * P:tiles[0] * P + rows,
                                          (b % 2) * P:(b % 2 + 1) * P],
                                in_=o32[:rows, :])
                        continue

                    for t in tiles:
                        nch_t = int(K[t].sum())
                        agg = accp.tile([P, P], f32, space="PSUM")
                        jj = 0
                        for b in range(NBANK):
                            for j in range(int(K[t, b])):
                                gc = int(col_base[t, b]) + j
                                m, s_lo = bufs[b]
                                lo = int(slot_base[t, b]) - s_lo + j
                                sc = gc - rch0
                                nc.tensor.matmul(
                                    out=agg[:],
                                    lhsT=m[:, lo * D:(lo + 1) * D],
                                    rhs=s_sb[:, sc * P:(sc + 1) * P],
                                    start=(jj == 0),
                                    stop=(jj == nch_t - 1),
                                )
                                jj += 1

                        # epilogue: aggT[f_in, d] -> GEMM -> +bias -> relu
                        aggT = epool.tile([P, P], fp16, tag="aggT")
                        nc.scalar.activation(out=aggT[:], in_=agg[:],
                                             func=mybir.ActivationFunctionType.Copy)
                        po = gemp.tile([P, P], f32, space="PSUM")
                        nc.tensor.matmul(out=po[:], lhsT=aggT[:], rhs=w_sb[li][:],
                                         start=True, stop=True)
                        t1 = epool.tile([P, P], f32, tag="t1")
                        nc.vector.tensor_scalar(out=t1[:], in0=po[:],
                                                scalar1=dinvp_sb[:, t:t + 1],
                                                scalar2=None,
                                                op0=mybir.AluOpType.mult)
                        t2 = epool.tile([P, P], f32, tag="t2")
                        nc.vector.tensor_tensor(out=t2[:], in0=t1[:],
                                                in1=b_sb[li][:],
                                                op=mybir.AluOpType.add)
                        rows = min(P, NLOC - t * P)
                        o32 = epool.tile([P, P], f32, tag="o32")
                        nc.scalar.activation(out=o32[:], in_=t2[:],
                                             func=mybir.ActivationFunctionType.Relu)
                        nc.scalar.dma_start(
                            out=out_d[t * P:t * P + rows, li * D:(li + 1) * D],
                            in_=o32[:rows, :])
                        if li == 0:
                            h1s_t = epool.tile([P, P], fp16, tag="h1s")
                            nc.scalar.activation(
                                out=h1s_t[:], in_=t2[:],
                                func=mybir.ActivationFunctionType.Relu,
                                scale=dinvp_sb[:, t:t + 1])
                            nc.scalar.dma_start(
                                out=h1s_loc[t * P:t * P + rows, :],
                                in_=h1s_t[:rows, :])

            layer(0, xs)

            if ablate != "l1":
                nc.gpsimd.collective_compute(
                    "AllGather",
                    mybir.AluOpType.bypass,
                    replica_groups=[list(range(NC))],
                    ins=[h1s_loc[:, :]],
                    outs=[h1s_full[:, :]],
                )
                tc.strict_bb_all_engine_barrier()

                layer(1, h1s_full)

    nc.finalize()
    return nc


def _run(inputs, trace=False, trace_kwargs=None):
    from concourse.bass_utils import run_bass_kernel_spmd

    x = np.asarray(inputs["x"], np.float32)
    edge_index = np.asarray(inputs["edge_index"])
    W1 = np.asarray(inputs["W1"], np.float32)
    b1 = np.asarray(inputs["b1"], np.float32)
    W2 = np.asarray(inputs["W2"], np.float32)
    b2 = np.asarray(inputs["b2"], np.float32)

    sched, percore, dinv = _schedule(edge_index)

    key = (sched["c_total"], tuple(sched["K"].ravel().tolist()))
    if key not in _cache:
        _cache.clear()
        _cache[key] = _build_program(sched)
    nc = _cache[key]

    xs_np = (x * dinv[:, None]).astype(np.float16)
    w1_np = W1.astype(np.float16)
    w2_np = W2.astype(np.float16)
    b1_np = np.tile(b1.astype(np.float32), (P, 1))
    b2_np = np.tile(b2.astype(np.float32), (P, 1))

    in_maps = []
    for c in range(NC):
        in_maps.append({
            "xs": xs_np,
            "idx": percore["idx"][c],
            "s8": percore["s"][c],
            "dinvp": percore["dinvp"][c],
            "w1": w1_np,
            "w2": w2_np,
            "b1": b1_np,
            "b2": b2_np,
        })

    res = run_bass_kernel_spmd(nc, in_maps, list(range(NC)), trace=trace,
                               **(trace_kwargs or {}))
    out = np.concatenate([res.results[c]["out"] for c in range(NC)], axis=0)
    return out, res


def kernel(x, edge_index, W1, b1, W2, b2):
    out, _ = _run(dict(x=x, edge_index=edge_index, W1=W1, b1=b1, W2=W2, b2=b2))
    return out


# revision 18
# speedup vs baseline: 1.3458x; 1.3458x over previous
"""GCN encoder (2-layer) on 8 trn2 NeuronCores via Bass/Tile.

Strategy (sharding_hint: 1D partition nodes by destination):
  - Nodes sharded by destination: core c owns rows [c*12500, (c+1)*12500).
  - Per layer, messages are gathered per-edge with dma_gather (SWDGE) from a
    full replica of the (scaled) source features in HBM, combined into
    per-destination sums with PE matmuls against one-hot selection matrices
    (built on DVE with is_equal), then the dense 128x128 GEMM runs on the
    aggregated features (GEMM commutes with the segment-sum).
  - Layer 1 sources = dinv-scaled x (host-prepared, replicated, no comms).
    Layer 2 sources = dinv-scaled h1, exchanged with ONE AllGather.
  - dma_gather indices are int16, so sources are split into 4 banks
    (32768, 32768, 32768, 1696 rows); edges are grouped per (dest-tile, bank)
    and padded to 128-edge chunks, with chunk counts maxed over cores so the
    same program (SPMD) runs on all 8 cores.
"""

import sys
import numpy as np

sys.path.insert(0, "/opt/trn_rl_repo")
sys.path.insert(0, "/opt/trn_rl_repo/concourse")

N, E, D = 100000, 1600000, 128
NC = 8
NLOC = N // NC            # 12500 rows per core
P = 128
NT = (NLOC + P - 1) // P  # 98 dest tiles per core (last tile 84 rows)
REG = 4                   # dest tiles per gather region
BANK_BOUNDS = [0, 32768, 65536, 98304, N]
NBANK = 4

_cache = {}


def _schedule(edge_index):
    """Compute the cross-core-uniform chunk schedule and per-core arrays."""
    row = edge_index[0].astype(np.int64)
    col = edge_index[1].astype(np.int64)
    loop = np.arange(N, dtype=np.int64)
    row = np.concatenate([row, loop])
    col = np.concatenate([col, loop])

    deg = np.bincount(col, minlength=N).astype(np.float32)
    dinv = np.where(deg > 0, 1.0 / np.sqrt(deg), 0.0).astype(np.float32)

    core = col // NLOC
    tile = (col % NLOC) // P
    bank = np.searchsorted(BANK_BOUNDS, row, side="right") - 1
    key = (core * NT + tile) * NBANK + bank
    order = np.argsort(key, kind="stable")
    row_s, col_s, key_s = row[order], col[order], key[order]

    counts = np.bincount(key_s, minlength=NC * NT * NBANK).reshape(NC, NT, NBANK)
    K = np.ceil(counts.max(axis=0) / P).astype(np.int64)      # [NT, NBANK]
    K[:, 0] = np.maximum(K[:, 0], 1)                           # every tile has >=1 chunk

    # chunk columns in matmul order: (region, tile, bank, j)
    col_base = np.zeros((NT, NBANK), np.int64)
    c_total = 0
    for t in range(NT):
        for b in range(NBANK):
            col_base[t, b] = c_total
            c_total += K[t, b]

    # gather-call layout: per region r, per bank b, chunks for tiles of r in
    # t order.  slot_base[t, b] = slot index (in 128-edge units) of chunk 0 of
    # (t, b) within the global call stream.
    regions = [list(range(r, min(r + REG, NT))) for r in range(0, NT, REG)]
    CALL_CH = 12              # chunks per dma_gather sub-call (1536 indices)
    region_calls = []         # per region: list of (bank, slot_chunk_lo, n_chunks)
    slot_base = np.zeros((NT, NBANK), np.int64)
    stream_pos = 0
    for tiles in regions:
        rc = []
        for b in range(NBANK):
            n_ch = int(sum(K[t, b] for t in tiles))
            if n_ch == 0:
                continue
            for t in tiles:
                slot_base[t, b] = stream_pos
                stream_pos += K[t, b]
            lo = stream_pos - n_ch
            # split into uniform sub-calls so SWDGE queue contexts pipeline
            nsub = max(1, (n_ch + CALL_CH - 1) // CALL_CH)
            base = n_ch // nsub
            rem = n_ch - base * nsub
            off = lo
            for si in range(nsub):
                cnt = base + (1 if si < rem else 0)
                rc.append((b, off, cnt))
                off += cnt
            assert off == stream_pos
        region_calls.append(rc)
    assert stream_pos == c_total

    tot_slots = c_total * P

    # per-core arrays
    group_cnt = counts  # [NC, NT, NBANK]
    group_off = np.zeros(NC * NT * NBANK + 1, np.int64)
    np.cumsum(np.bincount(key_s, minlength=NC * NT * NBANK), out=group_off[1:])

    import ml_dtypes
    idx_list, s_list = [], []
    for c in range(NC):
        idx16 = np.zeros((P, tot_slots // 16), np.int16)
        s_flat = np.zeros(P * c_total * P, np.uint8)  # fp8 one-hot, built flat
        one_fp8 = np.float32(1.0).astype(ml_dtypes.float8_e4m3).view(np.uint8)
        for t in range(NT):
            for b in range(NBANK):
                g = (c * NT + t) * NBANK + b
                lo, hi = group_off[g], group_off[g + 1]
                n = hi - lo
                if n == 0 and K[t, b] == 0:
                    continue
                pos = np.arange(n)
                j = pos // P
                p = pos % P
                # matmul-order one-hot: S[p, (col_base+j)*128 + dloc] = 1
                cols = col_base[t, b] + j
                dloc = col_s[lo:hi] - (c * NLOC + t * P)
                s_flat[(p * c_total + cols) * P + dloc] = one_fp8
                # gather stream (idx pads are 0 -> harmless row-0 gathers)
                slot = (slot_base[t, b] + j) * P + p
                v = (row_s[lo:hi] - BANK_BOUNDS[b]).astype(np.int16)
                idx16[slot % 16, slot // 16] = v
        idx16 = np.tile(idx16[:16], (8, 1))
        idx_list.append(idx16)
        s_list.append(s_flat.view(ml_dtypes.float8_e4m3).reshape(P, c_total * P))

    dinvp = np.zeros((P, NT), np.float32)
    dinvp_all = []
    for c in range(NC):
        dp = np.zeros((P, NT), np.float32)
        dl = dinv[c * NLOC:(c + 1) * NLOC]
        dl = np.pad(dl, (0, NT * P - NLOC))
        dp[:, :] = dl.reshape(NT, P).T
        dinvp_all.append(dp)
    del dinvp

    sched = {
        "K": K, "col_base": col_base, "slot_base": slot_base,
        "region_calls": region_calls, "regions": regions,
        "c_total": c_total, "tot_slots": tot_slots,
    }
    percore = {
        "idx": idx_list, "s": s_list, "dinvp": dinvp_all,
    }
    return sched, percore, dinv


def _build_program(sched):
    import os
    from concourse import bass, bacc, mybir
    import concourse.tile as tile

    ablate = os.environ.get("GCN_ABLATE", "")

    fp16 = mybir.dt.float16
    f32 = mybir.dt.float32
    i16 = mybir.dt.int16

    K = sched["K"]
    col_base = sched["col_base"]
    slot_base = sched["slot_base"]
    region_calls = sched["region_calls"]
    regions = sched["regions"]
    c_total = sched["c_total"]
    tot_slots = sched["tot_slots"]

    nc = bacc.Bacc("TRN2", target_bir_lowering=False, debug=False,
                   num_devices=NC, num_swdge_queues=4)

    xs = nc.declare_dram_parameter("xs", [N, D], fp16, isOutput=False)
    idx = nc.declare_dram_parameter("idx", [P, tot_slots // 16], i16, isOutput=False)
    s_d = nc.declare_dram_parameter("s8", [P, c_total * P], mybir.dt.float8e4,
                                    isOutput=False)
    dinvp_d = nc.declare_dram_parameter("dinvp", [P, NT], f32, isOutput=False)
    w1_d = nc.declare_dram_parameter("w1", [D, D], fp16, isOutput=False)
    w2_d = nc.declare_dram_parameter("w2", [D, D], fp16, isOutput=False)
    b1_d = nc.declare_dram_parameter("b1", [P, D], f32, isOutput=False)
    b2_d = nc.declare_dram_parameter("b2", [P, D], f32, isOutput=False)
    out_d = nc.declare_dram_parameter("out", [NLOC, 2 * D], f32, isOutput=True)

    h1s_loc = nc.dram_tensor("h1s_loc", [NLOC, D], fp16)
    h1s_full = nc.dram_tensor("h1s_full", [N, D], fp16, addr_space="Shared")

    with tile.TileContext(nc) as tc:
        with tc.tile_pool(name="const", bufs=1) as cpool, \
             tc.tile_pool(name="msg", bufs=4) as mpool, \
             tc.tile_pool(name="sel", bufs=3) as spool, \
             tc.tile_pool(name="epi", bufs=3) as epool, \
             tc.tile_pool(name="acc", bufs=3, space="PSUM") as accp, \
             tc.tile_pool(name="gemm", bufs=2, space="PSUM") as gemp:

            idx_sb = cpool.tile([P, tot_slots // 16], i16)
            nc.sync.dma_start(out=idx_sb[:], in_=idx[:, :])
            dinvp_sb = cpool.tile([P, NT], f32)
            nc.sync.dma_start(out=dinvp_sb[:], in_=dinvp_d[:, :])
            w_sb = [cpool.tile([P, D], fp16, name="w1sb"),
                    cpool.tile([P, D], fp16, name="w2sb")]
            nc.sync.dma_start(out=w_sb[0][:], in_=w1_d[:, :])
            nc.sync.dma_start(out=w_sb[1][:], in_=w2_d[:, :])
            b_sb = [cpool.tile([P, D], f32, name="b1sb"),
                    cpool.tile([P, D], f32, name="b2sb")]
            nc.sync.dma_start(out=b_sb[0][:], in_=b1_d[:, :])
            nc.sync.dma_start(out=b_sb[1][:], in_=b2_d[:, :])

            qi = [0]

            def layer(li, src):
                for ri, tiles in enumerate(regions):
                    # load this region's selection matrices (fp8 one-hot)
                    rch0 = int(col_base[tiles[0], 0])
                    rchn = int(sum(K[t].sum() for t in tiles))
                    s_sb = spool.tile([P, rchn * P], mybir.dt.float8e4, tag="S")
                    nc.sync.dma_start(out=s_sb[:],
                                      in_=s_d[:, rch0 * P:(rch0 + rchn) * P])
                    # gather this region's chunks (uniform sub-calls,
                    # strict queue round-robin so the 4 SWDGE contexts
                    # generate descriptors in parallel)
                    bufs = {}
                    bank_lo = {}
                    for b, s_lo, n_ch in region_calls[ri]:
                        if b not in bufs:
                            tot_b = sum(c for bb, _, c in region_calls[ri]
                                        if bb == b)
                            bufs[b] = mpool.tile([P, tot_b * D], fp16,
                                                 name=f"msgb{b}",
                                                 tag=f"msg{b}")
                            bank_lo[b] = s_lo
                        m = bufs[b]
                        mo = s_lo - bank_lo[b]
                        nc.gpsimd.dma_gather(
                            out_ap=m[:, mo * D:(mo + n_ch) * D].rearrange(
                                "p (c f) -> p c f", f=D),
                            in_ap=src[BANK_BOUNDS[b]:BANK_BOUNDS[b + 1], :],
                            idxs_ap=idx_sb[:, s_lo * 8:(s_lo + n_ch) * 8],
                            num_idxs=n_ch * P,
                            num_idxs_reg=n_ch * P,
                            elem_size=D,
                            single_packet=False,
                            queue_num=1 + qi[0] % 3,
                        )
                        qi[0] += 1
                    bufs = {b: (m, bank_lo[b]) for b, m in bufs.items()}

                    if ablate == "gather":
                        # consume msg buffers minimally, skip seg/epilogue
                        for b, (m, s_lo) in bufs.items():
                            o32 = epool.tile([P, P], f32, tag="o32")
                            nc.vector.tensor_copy(out=o32[:], in_=m[:, :P])
                            rows = min(P, NLOC - tiles[0] * P)
                            nc.sync.dma_start(
                                out=out_d[tiles[0] * P:tiles[0] * P + rows,
                                          (b % 2) * P:(b % 2 + 1) * P],
                                in_=o32[:rows, :])
                        continue

                    for t in tiles:
                        nch_t = int(K[t].sum())
                        agg = accp.tile([P, P], f32, space="PSUM")
                        jj = 0
                        for b in range(NBANK):
                            for j in range(int(K[t, b])):
                                gc = int(col_base[t, b]) + j
                                m, s_lo = bufs[b]
                                lo = int(slot_base[t, b]) - s_lo + j
                                sc = gc - rch0
                                nc.tensor.matmul(
                                    out=agg[:],
                                    lhsT=m[:, lo * D:(lo + 1) * D],
                                    rhs=s_sb[:, sc * P:(sc + 1) * P],
                                    start=(jj == 0),
                                    stop=(jj == nch_t - 1),
                                )
                                jj += 1

                        # epilogue: aggT[f_in, d] -> GEMM -> +bias -> relu
                        aggT = epool.tile([P, P], fp16, tag="aggT")
                        nc.scalar.activation(out=aggT[:], in_=agg[:],
                                             func=mybir.ActivationFunctionType.Copy)
                        po = gemp.tile([P, P], f32, space="PSUM")
                        nc.tensor.matmul(out=po[:], lhsT=aggT[:], rhs=w_sb[li][:],
                                         start=True, stop=True)
                        t1 = epool.tile([P, P], f32, tag="t1")
                        nc.vector.tensor_scalar(out=t1[:], in0=po[:],
                                                scalar1=dinvp_sb[:, t:t + 1],
                                                scalar2=None,
                                                op0=mybir.AluOpType.mult)
                        t2 = epool.tile([P, P], f32, tag="t2")
                        nc.vector.tensor_tensor(out=t2[:], in0=t1[:],
                                                in1=b_sb[li][:],
                                                op=mybir.AluOpType.add)
                        rows = min(P, NLOC - t * P)
                        o32 = epool.tile([P, P], f32, tag="o32")
                        nc.scalar.activation(out=o32[:], in_=t2[:],
                                             func=mybir.ActivationFunctionType.Relu)
                        nc.sync.dma_start(
                            out=out_d[t * P:t * P + rows, li * D:(li + 1) * D],
                            in_=o32[:rows, :])
                        if li == 0:
                            h1s_t = epool.tile([P, P], fp16, tag="h1s")
                            nc.scalar.activation(
                                out=h1s_t[:], in_=t2[:],
                                func=mybir.ActivationFunctionType.Relu,
                                scale=dinvp_sb[:, t:t + 1])
                            nc.sync.dma_start(
                                out=h1s_loc[t * P:t * P + rows, :],
                                in_=h1s_t[:rows, :])

            layer(0, xs)

            if ablate != "l1":
                nc.gpsimd.collective_compute(
                    "AllGather",
                    mybir.AluOpType.bypass,
                    replica_groups=[list(range(NC))],
                    ins=[h1s_loc[:, :]],
                    outs=[h1s_full[:, :]],
                )
                tc.strict_bb_all_engine_barrier()

                layer(1, h1s_full)

    nc.finalize()
    return nc


def _run(inputs, trace=False, trace_kwargs=None):
    from concourse.bass_utils import run_bass_kernel_spmd

    x = np.asarray(inputs["x"], np.float32)
    edge_index = np.asarray(inputs["edge_index"])
    W1 = np.asarray(inputs["W1"], np.float32)
    b1 = np.asarray(inputs["b1"], np.float32)
    W2 = np.asarray(inputs["W2"], np.float32)
    b2 = np.asarray(inputs["b2"], np.float32)

    sched, percore, dinv = _schedule(edge_index)

    key = (sched["c_total"], tuple(sched["K"].ravel().tolist()))
    if key not in _cache:
        _cache.clear()
        _cache[key] = _build_program(sched)
    nc = _cache[key]

    xs_np = (x * dinv[:, None]).astype(np.float16)
    w1_np = W1.astype(np.float16)
    w2_np = W2.astype(np.float16)
    b1_np = np.tile(b1.astype(np.float32), (P, 1))
    b2_np = np.tile(b2.astype(np.float32), (P, 1))

    in_maps = []
    for c in range(NC):
        in_maps.append({
            "xs": xs_np,
            "idx": percore["idx"][c],
            "s8": percore["s"][c],
            "dinvp": percore["dinvp"][c],
            "w1": w1_np,
            "w2": w2_np,
            "b1": b1_np,
            "b2": b2_np,
        })

    res = run_bass_kernel_spmd(nc, in_maps, list(range(NC)), trace=trace,
                               **(trace_kwargs or {}))
    out = np.concatenate([res.results[c]["out"] for c in range(NC)], axis=0)
    return out, res


def kernel(x, edge_index, W1, b1, W2, b2):
    out, _ = _run(dict(x=x, edge_index=edge_index, W1=W1, b1=b1, W2=W2, b2=b2))
    return out


# revision 19
# speedup vs baseline: 1.6057x; 1.1931x over previous
"""GCN encoder (2-layer) on 8 trn2 NeuronCores via Bass/Tile.

Strategy (sharding_hint: 1D partition nodes by destination):
  - Nodes sharded by destination: core c owns rows [c*12500, (c+1)*12500).
  - Per layer, messages are gathered per-edge with dma_gather (SWDGE) from a
    full replica of the (scaled) source features in HBM, combined into
    per-destination sums with PE matmuls against one-hot selection matrices
    (built on DVE with is_equal), then the dense 128x128 GEMM runs on the
    aggregated features (GEMM commutes with the segment-sum).
  - Layer 1 sources = dinv-scaled x (host-prepared, replicated, no comms).
    Layer 2 sources = dinv-scaled h1, exchanged with ONE AllGather.
  - dma_gather indices are int16, so sources are split into 4 banks
    (32768, 32768, 32768, 1696 rows); edges are grouped per (dest-tile, bank)
    and padded to 128-edge chunks, with chunk counts maxed over cores so the
    same program (SPMD) runs on all 8 cores.
"""

import sys
import numpy as np

sys.path.insert(0, "/opt/trn_rl_repo")
sys.path.insert(0, "/opt/trn_rl_repo/concourse")

N, E, D = 100000, 1600000, 128
NC = 8
NLOC = N // NC            # 12500 rows per core
P = 128
NT = (NLOC + P - 1) // P  # 98 dest tiles per core (last tile 84 rows)
REG = 4                   # dest tiles per gather region
BANK_BOUNDS = [0, 32768, 65536, 98304, N]
NBANK = 4

_cache = {}


def _schedule(edge_index):
    """Compute the cross-core-uniform chunk schedule and per-core arrays."""
    row = edge_index[0].astype(np.int64)
    col = edge_index[1].astype(np.int64)
    loop = np.arange(N, dtype=np.int64)
    row = np.concatenate([row, loop])
    col = np.concatenate([col, loop])

    deg = np.bincount(col, minlength=N).astype(np.float32)
    dinv = np.where(deg > 0, 1.0 / np.sqrt(deg), 0.0).astype(np.float32)

    core = col // NLOC
    tile = (col % NLOC) // P
    bank = np.searchsorted(BANK_BOUNDS, row, side="right") - 1
    key = (core * NT + tile) * NBANK + bank
    order = np.argsort(key, kind="stable")
    row_s, col_s, key_s = row[order], col[order], key[order]

    counts = np.bincount(key_s, minlength=NC * NT * NBANK).reshape(NC, NT, NBANK)
    K = np.ceil(counts.max(axis=0) / P).astype(np.int64)      # [NT, NBANK]
    K[:, 0] = np.maximum(K[:, 0], 1)                           # every tile has >=1 chunk

    # chunk columns in matmul order: (region, tile, bank, j)
    col_base = np.zeros((NT, NBANK), np.int64)
    c_total = 0
    for t in range(NT):
        for b in range(NBANK):
            col_base[t, b] = c_total
            c_total += K[t, b]

    # gather-call layout: per region r, per bank b, chunks for tiles of r in
    # t order.  slot_base[t, b] = slot index (in 128-edge units) of chunk 0 of
    # (t, b) within the global call stream.
    regions = [list(range(r, min(r + REG, NT))) for r in range(0, NT, REG)]
    CALL_CH = 12              # chunks per dma_gather sub-call (1536 indices)
    region_calls = []         # per region: list of (bank, slot_chunk_lo, n_chunks)
    slot_base = np.zeros((NT, NBANK), np.int64)
    stream_pos = 0
    for tiles in regions:
        rc = []
        for b in range(NBANK):
            n_ch = int(sum(K[t, b] for t in tiles))
            if n_ch == 0:
                continue
            for t in tiles:
                slot_base[t, b] = stream_pos
                stream_pos += K[t, b]
            lo = stream_pos - n_ch
            # split into uniform sub-calls so SWDGE queue contexts pipeline
            nsub = max(1, (n_ch + CALL_CH - 1) // CALL_CH)
            base = n_ch // nsub
            rem = n_ch - base * nsub
            off = lo
            for si in range(nsub):
                cnt = base + (1 if si < rem else 0)
                rc.append((b, off, cnt))
                off += cnt
            assert off == stream_pos
        region_calls.append(rc)
    assert stream_pos == c_total

    tot_slots = c_total * P

    # per-core arrays
    group_cnt = counts  # [NC, NT, NBANK]
    group_off = np.zeros(NC * NT * NBANK + 1, np.int64)
    np.cumsum(np.bincount(key_s, minlength=NC * NT * NBANK), out=group_off[1:])

    import ml_dtypes
    idx_list, s_list = [], []
    for c in range(NC):
        idx16 = np.zeros((P, tot_slots // 16), np.int16)
        s_flat = np.zeros(P * c_total * P, np.uint8)  # fp8 one-hot, built flat
        one_fp8 = np.float32(1.0).astype(ml_dtypes.float8_e4m3).view(np.uint8)
        for t in range(NT):
            for b in range(NBANK):
                g = (c * NT + t) * NBANK + b
                lo, hi = group_off[g], group_off[g + 1]
                n = hi - lo
                if n == 0 and K[t, b] == 0:
                    continue
                pos = np.arange(n)
                j = pos // P
                p = pos % P
                # matmul-order one-hot: S[p, (col_base+j)*128 + dloc] = 1
                cols = col_base[t, b] + j
                dloc = col_s[lo:hi] - (c * NLOC + t * P)
                s_flat[(p * c_total + cols) * P + dloc] = one_fp8
                # gather stream (idx pads are 0 -> harmless row-0 gathers)
                slot = (slot_base[t, b] + j) * P + p
                v = (row_s[lo:hi] - BANK_BOUNDS[b]).astype(np.int16)
                idx16[slot % 16, slot // 16] = v
        idx16 = np.tile(idx16[:16], (8, 1))
        idx_list.append(idx16)
        s_list.append(s_flat.view(ml_dtypes.float8_e4m3).reshape(P, c_total * P))

    dinvp = np.zeros((P, NT), np.float32)
    dinvp_all = []
    for c in range(NC):
        dp = np.zeros((P, NT), np.float32)
        dl = dinv[c * NLOC:(c + 1) * NLOC]
        dl = np.pad(dl, (0, NT * P - NLOC))
        dp[:, :] = dl.reshape(NT, P).T
        dinvp_all.append(dp)
    del dinvp

    sched = {
        "K": K, "col_base": col_base, "slot_base": slot_base,
        "region_calls": region_calls, "regions": regions,
        "c_total": c_total, "tot_slots": tot_slots,
    }
    percore = {
        "idx": idx_list, "s": s_list, "dinvp": dinvp_all,
    }
    return sched, percore, dinv


def _build_program(sched):
    import os
    from concourse import bass, bacc, mybir
    import concourse.tile as tile

    ablate = os.environ.get("GCN_ABLATE", "")

    fp16 = mybir.dt.float16
    f32 = mybir.dt.float32
    i16 = mybir.dt.int16

    K = sched["K"]
    col_base = sched["col_base"]
    slot_base = sched["slot_base"]
    region_calls = sched["region_calls"]
    regions = sched["regions"]
    c_total = sched["c_total"]
    tot_slots = sched["tot_slots"]

    nc = bacc.Bacc("TRN2", target_bir_lowering=False, debug=False,
                   num_devices=NC, num_swdge_queues=4)

    xs = nc.declare_dram_parameter("xs", [N, D], fp16, isOutput=False)
    idx = nc.declare_dram_parameter("idx", [P, tot_slots // 16], i16, isOutput=False)
    s_d = nc.declare_dram_parameter("s8", [P, c_total * P], mybir.dt.float8e4,
                                    isOutput=False)
    dinvp_d = nc.declare_dram_parameter("dinvp", [P, NT], f32, isOutput=False)
    w1_d = nc.declare_dram_parameter("w1", [D, D], fp16, isOutput=False)
    w2_d = nc.declare_dram_parameter("w2", [D, D], fp16, isOutput=False)
    b1_d = nc.declare_dram_parameter("b1", [P, D], f32, isOutput=False)
    b2_d = nc.declare_dram_parameter("b2", [P, D], f32, isOutput=False)
    out_d = nc.declare_dram_parameter("out", [NLOC, 2 * D], f32, isOutput=True)

    h1s_loc = nc.dram_tensor("h1s_loc", [NLOC, D], fp16)
    h1s_full = nc.dram_tensor("h1s_full", [N, D], fp16, addr_space="Shared")
    h1s_fl = nc.dram_tensor("h1s_fl", [N, D], fp16)

    with tile.TileContext(nc) as tc:
        with tc.tile_pool(name="const", bufs=1) as cpool, \
             tc.tile_pool(name="msg", bufs=4) as mpool, \
             tc.tile_pool(name="sel", bufs=3) as spool, \
             tc.tile_pool(name="epi", bufs=3) as epool, \
             tc.tile_pool(name="acc", bufs=3, space="PSUM") as accp, \
             tc.tile_pool(name="gemm", bufs=2, space="PSUM") as gemp:

            idx_sb = cpool.tile([P, tot_slots // 16], i16)
            nc.sync.dma_start(out=idx_sb[:], in_=idx[:, :])
            dinvp_sb = cpool.tile([P, NT], f32)
            nc.sync.dma_start(out=dinvp_sb[:], in_=dinvp_d[:, :])
            w_sb = [cpool.tile([P, D], fp16, name="w1sb"),
                    cpool.tile([P, D], fp16, name="w2sb")]
            nc.sync.dma_start(out=w_sb[0][:], in_=w1_d[:, :])
            nc.sync.dma_start(out=w_sb[1][:], in_=w2_d[:, :])
            b_sb = [cpool.tile([P, D], f32, name="b1sb"),
                    cpool.tile([P, D], f32, name="b2sb")]
            nc.sync.dma_start(out=b_sb[0][:], in_=b1_d[:, :])
            nc.sync.dma_start(out=b_sb[1][:], in_=b2_d[:, :])

            qi = [0]

            def layer(li, src):
                for ri, tiles in enumerate(regions):
                    # load this region's selection matrices (fp8 one-hot)
                    rch0 = int(col_base[tiles[0], 0])
                    rchn = int(sum(K[t].sum() for t in tiles))
                    s_sb = spool.tile([P, rchn * P], mybir.dt.float8e4, tag="S")
                    nc.sync.dma_start(out=s_sb[:],
                                      in_=s_d[:, rch0 * P:(rch0 + rchn) * P])
                    # gather this region's chunks (uniform sub-calls,
                    # strict queue round-robin so the 4 SWDGE contexts
                    # generate descriptors in parallel)
                    bufs = {}
                    bank_lo = {}
                    for b, s_lo, n_ch in region_calls[ri]:
                        if b not in bufs:
                            tot_b = sum(c for bb, _, c in region_calls[ri]
                                        if bb == b)
                            bufs[b] = mpool.tile([P, tot_b * D], fp16,
                                                 name=f"msgb{b}",
                                                 tag=f"msg{b}")
                            bank_lo[b] = s_lo
                        m = bufs[b]
                        mo = s_lo - bank_lo[b]
                        nc.gpsimd.dma_gather(
                            out_ap=m[:, mo * D:(mo + n_ch) * D].rearrange(
                                "p (c f) -> p c f", f=D),
                            in_ap=src[BANK_BOUNDS[b]:BANK_BOUNDS[b + 1], :],
                            idxs_ap=idx_sb[:, s_lo * 8:(s_lo + n_ch) * 8],
                            num_idxs=n_ch * P,
                            num_idxs_reg=n_ch * P,
                            elem_size=D,
                            single_packet=False,
                            queue_num=1 + qi[0] % 3,
                        )
                        qi[0] += 1
                    bufs = {b: (m, bank_lo[b]) for b, m in bufs.items()}

                    if ablate == "gather":
                        # consume msg buffers minimally, skip seg/epilogue
                        for b, (m, s_lo) in bufs.items():
                            o32 = epool.tile([P, P], f32, tag="o32")
                            nc.vector.tensor_copy(out=o32[:], in_=m[:, :P])
                            rows = min(P, NLOC - tiles[0] * P)
                            nc.sync.dma_start(
                                out=out_d[tiles[0] * P:tiles[0] * P + rows,
                                          (b % 2) * P:(b % 2 + 1) * P],
                                in_=o32[:rows, :])
                        continue

                    for t in tiles:
                        nch_t = int(K[t].sum())
                        agg = accp.tile([P, P], f32, space="PSUM")
                        jj = 0
                        for b in range(NBANK):
                            for j in range(int(K[t, b])):
                                gc = int(col_base[t, b]) + j
                                m, s_lo = bufs[b]
                                lo = int(slot_base[t, b]) - s_lo + j
                                sc = gc - rch0
                                nc.tensor.matmul(
                                    out=agg[:],
                                    lhsT=m[:, lo * D:(lo + 1) * D],
                                    rhs=s_sb[:, sc * P:(sc + 1) * P],
                                    start=(jj == 0),
                                    stop=(jj == nch_t - 1),
                                )
                                jj += 1

                        # epilogue: aggT[f_in, d] -> GEMM -> +bias -> relu
                        aggT = epool.tile([P, P], fp16, tag="aggT")
                        nc.scalar.activation(out=aggT[:], in_=agg[:],
                                             func=mybir.ActivationFunctionType.Copy)
                        po = gemp.tile([P, P], f32, space="PSUM")
                        nc.tensor.matmul(out=po[:], lhsT=aggT[:], rhs=w_sb[li][:],
                                         start=True, stop=True)
                        t1 = epool.tile([P, P], f32, tag="t1")
                        nc.vector.tensor_scalar(out=t1[:], in0=po[:],
                                                scalar1=dinvp_sb[:, t:t + 1],
                                                scalar2=None,
                                                op0=mybir.AluOpType.mult)
                        t2 = epool.tile([P, P], f32, tag="t2")
                        nc.vector.tensor_tensor(out=t2[:], in0=t1[:],
                                                in1=b_sb[li][:],
                                                op=mybir.AluOpType.add)
                        rows = min(P, NLOC - t * P)
                        o32 = epool.tile([P, P], f32, tag="o32")
                        nc.scalar.activation(out=o32[:], in_=t2[:],
                                             func=mybir.ActivationFunctionType.Relu)
                        nc.sync.dma_start(
                            out=out_d[t * P:t * P + rows, li * D:(li + 1) * D],
                            in_=o32[:rows, :])
                        if li == 0:
                            h1s_t = epool.tile([P, P], fp16, tag="h1s")
                            nc.scalar.activation(
                                out=h1s_t[:], in_=t2[:],
                                func=mybir.ActivationFunctionType.Relu,
                                scale=dinvp_sb[:, t:t + 1])
                            nc.sync.dma_start(
                                out=h1s_loc[t * P:t * P + rows, :],
                                in_=h1s_t[:rows, :])

            layer(0, xs)

            if ablate != "l1":
                nc.gpsimd.collective_compute(
                    "AllGather",
                    mybir.AluOpType.bypass,
                    replica_groups=[list(range(NC))],
                    ins=[h1s_loc[:, :]],
                    outs=[h1s_full[:, :]],
                )
                nc.sync.dma_start(out=h1s_fl[:, :], in_=h1s_full[:, :])
                tc.strict_bb_all_engine_barrier()

                layer(1, h1s_fl)

    nc.finalize()
    return nc


def _run(inputs, trace=False, trace_kwargs=None):
    from concourse.bass_utils import run_bass_kernel_spmd

    x = np.asarray(inputs["x"], np.float32)
    edge_index = np.asarray(inputs["edge_index"])
    W1 = np.asarray(inputs["W1"], np.float32)
    b1 = np.asarray(inputs["b1"], np.float32)
    W2 = np.asarray(inputs["W2"], np.float32)
    b2 = np.asarray(inputs["b2"], np.float32)

    sched, percore, dinv = _schedule(edge_index)

    key = (sched["c_total"], tuple(sched["K"].ravel().tolist()))
    if key not in _cache:
        _cache.clear()
        _cache[key] = _build_program(sched)
    nc = _cache[key]

    xs_np = (x * dinv[:, None]).astype(np.float16)
    w1_np = W1.astype(np.float16)
    w2_np = W2.astype(np.float16)
    b1_np = np.tile(b1.astype(np.float32), (P, 1))
    b2_np = np.tile(b2.astype(np.float32), (P, 1))

    in_maps = []
    for c in range(NC):
        in_maps.append({
            "xs": xs_np,
            "idx": percore["idx"][c],
            "s8": percore["s"][c],
            "dinvp": percore["dinvp"][c],
            "w1": w1_np,
            "w2": w2_np,
            "b1": b1_np,
            "b2": b2_np,
        })

    res = run_bass_kernel_spmd(nc, in_maps, list(range(NC)), trace=trace,
                               **(trace_kwargs or {}))
    out = np.concatenate([res.results[c]["out"] for c in range(NC)], axis=0)
    return out, res


def kernel(x, edge_index, W1, b1, W2, b2):
    out, _ = _run(dict(x=x, edge_index=edge_index, W1=W1, b1=b1, W2=W2, b2=b2))
    return out


# revision 21
# speedup vs baseline: 2.1609x; 1.3458x over previous
"""GCN encoder (2-layer) on 8 trn2 NeuronCores via Bass/Tile.

Strategy (sharding_hint: 1D partition nodes by destination):
  - Nodes sharded by destination: core c owns rows [c*12500, (c+1)*12500).
  - Per layer, messages are gathered per-edge with dma_gather (SWDGE) from a
    full replica of the (scaled) source features in HBM, combined into
    per-destination sums with PE matmuls against one-hot selection matrices
    (built on DVE with is_equal), then the dense 128x128 GEMM runs on the
    aggregated features (GEMM commutes with the segment-sum).
  - Layer 1 sources = dinv-scaled x (host-prepared, replicated, no comms).
    Layer 2 sources = dinv-scaled h1, exchanged with ONE AllGather.
  - dma_gather indices are int16, so sources are split into 4 banks
    (32768, 32768, 32768, 1696 rows); edges are grouped per (dest-tile, bank)
    and padded to 128-edge chunks, with chunk counts maxed over cores so the
    same program (SPMD) runs on all 8 cores.
"""

import sys
import numpy as np

sys.path.insert(0, "/opt/trn_rl_repo")
sys.path.insert(0, "/opt/trn_rl_repo/concourse")

N, E, D = 100000, 1600000, 128
NC = 8
NLOC = N // NC            # 12500 rows per core
P = 128
NT = (NLOC + P - 1) // P  # 98 dest tiles per core (last tile 84 rows)
REG = 4                   # dest tiles per gather region
BANK_BOUNDS = [0, 32768, 65536, 98304, N]
NBANK = 4

_cache = {}


def _schedule(edge_index):
    """Compute the cross-core-uniform chunk schedule and per-core arrays."""
    # self-loops are handled by per-tile identity chunks (plain DMA), not by
    # the SWDGE gather stream; only real edges are scheduled here.
    row = edge_index[0].astype(np.int64)
    col = edge_index[1].astype(np.int64)

    deg = (np.bincount(col, minlength=N) + 1).astype(np.float32)
    dinv = (1.0 / np.sqrt(deg)).astype(np.float32)

    core = col // NLOC
    tile = (col % NLOC) // P
    bank = np.searchsorted(BANK_BOUNDS, row, side="right") - 1
    key = (core * NT + tile) * NBANK + bank
    order = np.argsort(key, kind="stable")
    row_s, col_s, key_s = row[order], col[order], key[order]

    counts = np.bincount(key_s, minlength=NC * NT * NBANK).reshape(NC, NT, NBANK)
    K = np.ceil(counts.max(axis=0) / P).astype(np.int64)      # [NT, NBANK]
    K[:, 0] = np.maximum(K[:, 0], 1)                           # every tile has >=1 chunk

    # chunk columns in matmul order: (region, tile, bank, j)
    col_base = np.zeros((NT, NBANK), np.int64)
    c_total = 0
    for t in range(NT):
        for b in range(NBANK):
            col_base[t, b] = c_total
            c_total += K[t, b]

    # gather-call layout: per region r, per bank b, chunks for tiles of r in
    # t order.  slot_base[t, b] = slot index (in 128-edge units) of chunk 0 of
    # (t, b) within the global call stream.
    regions = [list(range(r, min(r + REG, NT))) for r in range(0, NT, REG)]
    CALL_CH = 12              # chunks per dma_gather sub-call (1536 indices)
    region_calls = []         # per region: list of (bank, slot_chunk_lo, n_chunks)
    slot_base = np.zeros((NT, NBANK), np.int64)
    stream_pos = 0
    for tiles in regions:
        rc = []
        for b in range(NBANK):
            n_ch = int(sum(K[t, b] for t in tiles))
            if n_ch == 0:
                continue
            for t in tiles:
                slot_base[t, b] = stream_pos
                stream_pos += K[t, b]
            lo = stream_pos - n_ch
            # split into uniform sub-calls so SWDGE queue contexts pipeline
            nsub = max(1, (n_ch + CALL_CH - 1) // CALL_CH)
            base = n_ch // nsub
            rem = n_ch - base * nsub
            off = lo
            for si in range(nsub):
                cnt = base + (1 if si < rem else 0)
                rc.append((b, off, cnt))
                off += cnt
            assert off == stream_pos
        region_calls.append(rc)
    assert stream_pos == c_total

    tot_slots = c_total * P

    # per-core arrays
    group_cnt = counts  # [NC, NT, NBANK]
    group_off = np.zeros(NC * NT * NBANK + 1, np.int64)
    np.cumsum(np.bincount(key_s, minlength=NC * NT * NBANK), out=group_off[1:])

    import ml_dtypes
    idx_list, s_list = [], []
    for c in range(NC):
        idx16 = np.zeros((P, tot_slots // 16), np.int16)
        s_flat = np.zeros(P * c_total * P, np.uint8)  # fp8 one-hot, built flat
        one_fp8 = np.float32(1.0).astype(ml_dtypes.float8_e4m3).view(np.uint8)
        for t in range(NT):
            for b in range(NBANK):
                g = (c * NT + t) * NBANK + b
                lo, hi = group_off[g], group_off[g + 1]
                n = hi - lo
                if n == 0 and K[t, b] == 0:
                    continue
                pos = np.arange(n)
                j = pos // P
                p = pos % P
                # matmul-order one-hot: S[p, (col_base+j)*128 + dloc] = 1
                cols = col_base[t, b] + j
                dloc = col_s[lo:hi] - (c * NLOC + t * P)
                s_flat[(p * c_total + cols) * P + dloc] = one_fp8
                # gather stream (idx pads are 0 -> harmless row-0 gathers)
                slot = (slot_base[t, b] + j) * P + p
                v = (row_s[lo:hi] - BANK_BOUNDS[b]).astype(np.int16)
                idx16[slot % 16, slot // 16] = v
        idx16 = np.tile(idx16[:16], (8, 1))
        idx_list.append(idx16)
        s_list.append(s_flat.view(ml_dtypes.float8_e4m3).reshape(P, c_total * P))

    dinvp = np.zeros((P, NT), np.float32)
    dinvp_all = []
    for c in range(NC):
        dp = np.zeros((P, NT), np.float32)
        dl = dinv[c * NLOC:(c + 1) * NLOC]
        dl = np.pad(dl, (0, NT * P - NLOC))
        dp[:, :] = dl.reshape(NT, P).T
        dinvp_all.append(dp)
    del dinvp

    sched = {
        "K": K, "col_base": col_base, "slot_base": slot_base,
        "region_calls": region_calls, "regions": regions,
        "c_total": c_total, "tot_slots": tot_slots,
    }
    percore = {
        "idx": idx_list, "s": s_list, "dinvp": dinvp_all,
    }
    return sched, percore, dinv


def _build_program(sched):
    import os
    from concourse import bass, bacc, mybir
    import concourse.tile as tile

    ablate = os.environ.get("GCN_ABLATE", "")

    fp16 = mybir.dt.float16
    f32 = mybir.dt.float32
    i16 = mybir.dt.int16

    K = sched["K"]
    col_base = sched["col_base"]
    slot_base = sched["slot_base"]
    region_calls = sched["region_calls"]
    regions = sched["regions"]
    c_total = sched["c_total"]
    tot_slots = sched["tot_slots"]

    nc = bacc.Bacc("TRN2", target_bir_lowering=False, debug=False,
                   num_devices=NC, num_swdge_queues=4)

    xs = nc.declare_dram_parameter("xs", [N, D], fp16, isOutput=False)
    idx = nc.declare_dram_parameter("idx", [P, tot_slots // 16], i16, isOutput=False)
    s_d = nc.declare_dram_parameter("s8", [P, c_total * P], mybir.dt.float8e4,
                                    isOutput=False)
    xloc_d = nc.declare_dram_parameter("xloc", [NLOC, D], fp16, isOutput=False)
    id_d = nc.declare_dram_parameter("id8", [P, P], mybir.dt.float8e4,
                                     isOutput=False)
    dinvp_d = nc.declare_dram_parameter("dinvp", [P, NT], f32, isOutput=False)
    w1_d = nc.declare_dram_parameter("w1", [D, D], fp16, isOutput=False)
    w2_d = nc.declare_dram_parameter("w2", [D, D], fp16, isOutput=False)
    b1_d = nc.declare_dram_parameter("b1", [P, D], f32, isOutput=False)
    b2_d = nc.declare_dram_parameter("b2", [P, D], f32, isOutput=False)
    out_d = nc.declare_dram_parameter("out", [NLOC, 2 * D], f32, isOutput=True)

    h1s_loc = nc.dram_tensor("h1s_loc", [NLOC, D], fp16)
    AGCH = [(0, 3200), (3200, 6400), (6400, 9600), (9600, NLOC)]
    h1s_ag = [nc.dram_tensor(f"h1s_ag{i}", [NC * (hi - lo), D], fp16,
                             addr_space="Shared")
              for i, (lo, hi) in enumerate(AGCH)]
    h1s_fl = nc.dram_tensor("h1s_fl", [N, D], fp16)

    with tile.TileContext(nc) as tc:
        with tc.tile_pool(name="const", bufs=1) as cpool, \
             tc.tile_pool(name="msg", bufs=4) as mpool, \
             tc.tile_pool(name="sel", bufs=3) as spool, \
             tc.tile_pool(name="epi", bufs=3) as epool, \
             tc.tile_pool(name="acc", bufs=3, space="PSUM") as accp, \
             tc.tile_pool(name="gemm", bufs=2, space="PSUM") as gemp:

            idx_sb = cpool.tile([P, tot_slots // 16], i16)
            nc.sync.dma_start(out=idx_sb[:], in_=idx[:, :])
            dinvp_sb = cpool.tile([P, NT], f32)
            nc.sync.dma_start(out=dinvp_sb[:], in_=dinvp_d[:, :])
            id_sb = cpool.tile([P, P], mybir.dt.float8e4)
            nc.sync.dma_start(out=id_sb[:], in_=id_d[:, :])
            w_sb = [cpool.tile([P, D], fp16, name="w1sb"),
                    cpool.tile([P, D], fp16, name="w2sb")]
            nc.sync.dma_start(out=w_sb[0][:], in_=w1_d[:, :])
            nc.sync.dma_start(out=w_sb[1][:], in_=w2_d[:, :])
            b_sb = [cpool.tile([P, D], f32, name="b1sb"),
                    cpool.tile([P, D], f32, name="b2sb")]
            nc.sync.dma_start(out=b_sb[0][:], in_=b1_d[:, :])
            nc.sync.dma_start(out=b_sb[1][:], in_=b2_d[:, :])

            qi = [0]

            def layer(li, src):
                for ri, tiles in enumerate(regions):
                    # load this region's selection matrices (fp8 one-hot)
                    rch0 = int(col_base[tiles[0], 0])
                    rchn = int(sum(K[t].sum() for t in tiles))
                    s_sb = spool.tile([P, rchn * P], mybir.dt.float8e4, tag="S")
                    nc.sync.dma_start(out=s_sb[:],
                                      in_=s_d[:, rch0 * P:(rch0 + rchn) * P])
                    # gather this region's chunks (uniform sub-calls,
                    # strict queue round-robin so the 4 SWDGE contexts
                    # generate descriptors in parallel)
                    bufs = {}
                    bank_lo = {}
                    for b, s_lo, n_ch in region_calls[ri]:
                        if b not in bufs:
                            tot_b = sum(c for bb, _, c in region_calls[ri]
                                        if bb == b)
                            bufs[b] = mpool.tile([P, tot_b * D], fp16,
                                                 name=f"msgb{b}",
                                                 tag=f"msg{b}")
                            bank_lo[b] = s_lo
                        m = bufs[b]
                        mo = s_lo - bank_lo[b]
                        nc.gpsimd.dma_gather(
                            out_ap=m[:, mo * D:(mo + n_ch) * D].rearrange(
                                "p (c f) -> p c f", f=D),
                            in_ap=src[BANK_BOUNDS[b]:BANK_BOUNDS[b + 1], :],
                            idxs_ap=idx_sb[:, s_lo * 8:(s_lo + n_ch) * 8],
                            num_idxs=n_ch * P,
                            num_idxs_reg=n_ch * P,
                            elem_size=D,
                            single_packet=False,
                            queue_num=1 + qi[0] % 3,
                        )
                        qi[0] += 1
                    bufs = {b: (m, bank_lo[b]) for b, m in bufs.items()}

                    if ablate == "gather":
                        # consume msg buffers minimally, skip seg/epilogue
                        for b, (m, s_lo) in bufs.items():
                            o32 = epool.tile([P, P], f32, tag="o32")
                            nc.vector.tensor_copy(out=o32[:], in_=m[:, :P])
                            rows = min(P, NLOC - tiles[0] * P)
                            nc.sync.dma_start(
                                out=out_d[tiles[0] * P:tiles[0] * P + rows,
                                          (b % 2) * P:(b % 2 + 1) * P],
                                in_=o32[:rows, :])
                        continue

                    for t in tiles:
                        nch_t = int(K[t].sum()) + 1
                        rows = min(P, NLOC - t * P)
                        agg = accp.tile([P, P], f32, space="PSUM")
                        selfsrc = xloc_d if li == 0 else h1s_loc
                        mself = mpool.tile([P, P], fp16, tag="mself")
                        nc.sync.dma_start(
                            out=mself[:rows, :],
                            in_=selfsrc[t * P:t * P + rows, :])
                        nc.tensor.matmul(out=agg[:], lhsT=mself[:rows, :],
                                         rhs=id_sb[:rows, :],
                                         start=True, stop=(nch_t == 1))
                        jj = 1
                        for b in range(NBANK):
                            for j in range(int(K[t, b])):
                                gc = int(col_base[t, b]) + j
                                m, s_lo = bufs[b]
                                lo = int(slot_base[t, b]) - s_lo + j
                                sc = gc - rch0
                                nc.tensor.matmul(
                                    out=agg[:],
                                    lhsT=m[:, lo * D:(lo + 1) * D],
                                    rhs=s_sb[:, sc * P:(sc + 1) * P],
                                    start=(jj == 0),
                                    stop=(jj == nch_t - 1),
                                )
                                jj += 1

                        # epilogue: aggT[f_in, d] -> GEMM -> +bias -> relu
                        aggT = epool.tile([P, P], fp16, tag="aggT")
                        nc.scalar.activation(out=aggT[:], in_=agg[:],
                                             func=mybir.ActivationFunctionType.Copy)
                        po = gemp.tile([P, P], f32, space="PSUM")
                        nc.tensor.matmul(out=po[:], lhsT=aggT[:], rhs=w_sb[li][:],
                                         start=True, stop=True)
                        t1 = epool.tile([P, P], f32, tag="t1")
                        nc.vector.tensor_scalar(out=t1[:], in0=po[:],
                                                scalar1=dinvp_sb[:, t:t + 1],
                                                scalar2=None,
                                                op0=mybir.AluOpType.mult)
                        t2 = epool.tile([P, P], f32, tag="t2")
                        nc.vector.tensor_tensor(out=t2[:], in0=t1[:],
                                                in1=b_sb[li][:],
                                                op=mybir.AluOpType.add)
                        o32 = epool.tile([P, P], f32, tag="o32")
                        nc.scalar.activation(out=o32[:], in_=t2[:],
                                             func=mybir.ActivationFunctionType.Relu)
                        nc.sync.dma_start(
                            out=out_d[t * P:t * P + rows, li * D:(li + 1) * D],
                            in_=o32[:rows, :])
                        if li == 0:
                            h1s_t = epool.tile([P, P], fp16, tag="h1s")
                            nc.scalar.activation(
                                out=h1s_t[:], in_=t2[:],
                                func=mybir.ActivationFunctionType.Relu,
                                scale=dinvp_sb[:, t:t + 1])
                            nc.sync.dma_start(
                                out=h1s_loc[t * P:t * P + rows, :],
                                in_=h1s_t[:rows, :])

            layer(0, xs)

            if ablate != "l1":
                fl_v = h1s_fl[:, :].rearrange("(c r) d -> c r d", c=NC)
                for i, (lo, hi) in enumerate(AGCH):
                    nc.gpsimd.collective_compute(
                        "AllGather",
                        mybir.AluOpType.bypass,
                        replica_groups=[list(range(NC))],
                        ins=[h1s_loc[lo:hi, :]],
                        outs=[h1s_ag[i][:, :]],
                    )
                    nc.sync.dma_start(
                        out=fl_v[:, lo:hi, :],
                        in_=h1s_ag[i][:, :].rearrange(
                            "(c r) d -> c r d", c=NC))
                tc.strict_bb_all_engine_barrier()

                layer(1, h1s_fl)

    nc.finalize()
    return nc


def _run(inputs, trace=False, trace_kwargs=None):
    from concourse.bass_utils import run_bass_kernel_spmd

    x = np.asarray(inputs["x"], np.float32)
    edge_index = np.asarray(inputs["edge_index"])
    W1 = np.asarray(inputs["W1"], np.float32)
    b1 = np.asarray(inputs["b1"], np.float32)
    W2 = np.asarray(inputs["W2"], np.float32)
    b2 = np.asarray(inputs["b2"], np.float32)

    sched, percore, dinv = _schedule(edge_index)

    key = (sched["c_total"], tuple(sched["K"].ravel().tolist()))
    if key not in _cache:
        _cache.clear()
        _cache[key] = _build_program(sched)
    nc = _cache[key]

    import ml_dtypes
    xs_np = (x * dinv[:, None]).astype(np.float16)
    id_np = np.eye(P, dtype=np.float32).astype(ml_dtypes.float8_e4m3)
    w1_np = W1.astype(np.float16)
    w2_np = W2.astype(np.float16)
    b1_np = np.tile(b1.astype(np.float32), (P, 1))
    b2_np = np.tile(b2.astype(np.float32), (P, 1))

    in_maps = []
    for c in range(NC):
        in_maps.append({
            "xs": xs_np,
            "xloc": xs_np[c * NLOC:(c + 1) * NLOC],
            "id8": id_np,
            "idx": percore["idx"][c],
            "s8": percore["s"][c],
            "dinvp": percore["dinvp"][c],
            "w1": w1_np,
            "w2": w2_np,
            "b1": b1_np,
            "b2": b2_np,
        })

    res = run_bass_kernel_spmd(nc, in_maps, list(range(NC)), trace=trace,
                               **(trace_kwargs or {}))
    out = np.concatenate([res.results[c]["out"] for c in range(NC)], axis=0)
    return out, res


def kernel(x, edge_index, W1, b1, W2, b2):
    out, _ = _run(dict(x=x, edge_index=edge_index, W1=W1, b1=b1, W2=W2, b2=b2))
    return out
